# revision 34
# baseline (speedup 1.0000x reference)
"""Trainium2 Bass kernel for nn_Attention_63745904608049.

Relative-position attention (TransformerXL-style shift, Enformer-style pos
basis), batch 1, seq 2048, d_model 1536, 8 heads x 64. Head-parallel over 8
NeuronCores.

v4c: [k, q]-oriented score pipeline. The relative shift AND the transpose
needed by the attn*V matmul are fused into ONE DGE transposed read of the
positional-score scratch in DRAM (dma_start_transpose from a sheared flat
AP). Content scores are computed directly transposed (K^T tiles stationary).
Softmax combine is multiplicative: anF = exp(T)^T * exp(C^T), with exp(T)
taken in bulk 2048-wide SBUF ops and exp(C) fused into the psum evacuation.
attn*V consumes the combined [k, q] tiles directly; softmax row sums ride a
ones-column in V. Host divides by row sums, sums partials over cores, adds
the output bias. No device collectives.

Self-contained: hardcodes shapes, builds one SPMD Bass graph, runs it on
cores 0-7 via run_bass_kernel_spmd, and reassembles the full output.
"""
import contextlib
import ctypes
import math
import os
import sys
import types

import numpy as np
import ml_dtypes

import concourse.bass as bass
import concourse.mybir as mybir
from concourse.tile import TileContext
from concourse.masks import make_identity
from concourse.bass_utils import run_bass_kernel_spmd

# ----------------------------------------------------------------------------
# problem constants
N = 2048
DM = 1536
H = 8
HD = 64
INNER = H * HD            # 512
NCORES = 8
QT = N // 128             # 16 query tiles
KT = N // 128             # 16 key tiles
NG = 4                    # 4 query groups of 512
WIN = 2175                # per-q-tile pos table window (128 + 2048 - 1)
TSTRIDE = 2304            # padded row stride of the T scratch (elements)
CHUNKS = [(0, 512), (512, 512), (1024, 512), (1536, 512), (2048, 127)]
F32 = mybir.dt.float32
BF16 = mybir.dt.bfloat16
FP16 = mybir.dt.float16

_LAST_RESULT = None       # BassKernelResults of the last run (for test.py)


# ----------------------------------------------------------------------------
# axon NTFF profiling hook (lets BASS_TRACE=1 produce exec_time_ns under axon)
def _install_ntff_hook(so_path="/opt/axon/libaxon_pjrt.so"):
    try:
        import antenv.axon_hooks  # noqa: F401
        return
    except ImportError:
        pass
    try:
        lib = ctypes.CDLL(so_path)
    except OSError:
        return
    if not hasattr(lib, "axon_start_nrt_profile"):
        return
    lib.axon_start_nrt_profile.argtypes = [ctypes.POINTER(ctypes.c_int64), ctypes.c_size_t]
    lib.axon_start_nrt_profile.restype = ctypes.c_int64
    lib.axon_stop_nrt_profile.argtypes = [ctypes.c_char_p]
    lib.axon_stop_nrt_profile.restype = ctypes.c_int64

    @contextlib.contextmanager
    def _hook(output_dir, device_ids):
        import jax
        jax.devices()
        if device_ids:
            ids = (ctypes.c_int64 * len(device_ids))(*device_ids)
            rc = lib.axon_start_nrt_profile(ids, len(device_ids))
        else:
            rc = lib.axon_start_nrt_profile(None, 0)
        if rc != 0:
            raise RuntimeError(f"axon_start_nrt_profile rc={rc}")
        try:
            yield
        finally:
            n = lib.axon_stop_nrt_profile(str(output_dir).encode())
            print(f"ntff profile: {n} file(s) written to {output_dir}")

    mod = types.ModuleType("antenv.axon_hooks")
    mod.get_axon_ntff_profile_hook = lambda: _hook
    mod.set_axon_ntff_profile_hook = lambda h: None
    sys.modules["antenv.axon_hooks"] = mod


_install_ntff_hook()


# ----------------------------------------------------------------------------
# BIR post-processing: this container's walrus build rejects instructions with
# more than one sync wait; split extra waits onto preceding NoOps.
def _split_waits(bir_bytes, maxw=1):
    import json
    d = json.loads(bir_bytes)
    counter = [0]
    for fn in d["functions"]:
        for blk in fn["blocks"]:
            out = []
            for ins in blk["instructions"]:
                si = ins.get("sync_info")
                waits = (si or {}).get("on_wait") or []
                if len(waits) > maxw:
                    excess = waits[:-maxw]
                    ins["sync_info"]["on_wait"] = waits[-maxw:]
                    for i in range(0, len(excess), maxw):
                        counter[0] += 1
                        nop = {
                            "engine": ins["engine"],
                            "ins": [],
                            "outs": [],
                            "name": f"I-waitsplit-{counter[0]}",
                            "opcode": "NoOp",
                            "sync_info": {"on_update": [],
                                          "on_wait": excess[i:i + maxw]},
                        }
                        if "debug" in ins:
                            nop["debug"] = ins["debug"]
                        out.append(nop)
                out.append(ins)
            blk["instructions"] = out
    return json.dumps(d).encode()


# ----------------------------------------------------------------------------
# host-side positional embedding table (pure function of N, DM)
_POS_CACHE = {}


def _pos_embed():
    if "emb" in _POS_CACHE:
        return _POS_CACHE["emb"]
    n, fs = N, DM
    nb = fs // 6  # 256
    dist = np.arange(-n + 1, n, dtype=np.float64)
    adist = np.abs(dist)[:, None]

    max_range = math.log(n) / math.log(2.0)
    half_life = 2.0 ** np.linspace(3.0, max_range, nb)
    exp_feat = np.exp(-math.log(2.0) / half_life[None, :] * adist)

    with np.errstate(over="ignore"):
        center_widths = 2.0 ** np.arange(1, nb + 1, dtype=np.float64) - 1.0
    cmask_feat = (center_widths[None, :] > adist).astype(np.float64)

    stddev = n / (2.0 * nb)
    start_mean = n / nb
    mean = np.linspace(start_mean, float(n), nb)[None, :]
    conc = (mean / stddev) ** 2
    rate = mean / stddev ** 2
    with np.errstate(divide="ignore", invalid="ignore"):
        log_unnorm = (conc - 1.0) * np.log(adist) - rate * adist
    lgamma = np.vectorize(math.lgamma)
    log_norm = lgamma(conc) - conc * np.log(rate)
    with np.errstate(invalid="ignore"):
        prob = np.exp(log_unnorm - log_norm) + 1e-08
    prob = np.nan_to_num(prob, nan=1e-08)  # adist=0: 0*inf -> use limit 0, then +eps
    zrow = np.where(adist[:, 0] == 0)[0]
    prob[zrow, :] = 1e-08
    gamma_feat = prob / prob.max(axis=-1, keepdims=True)

    emb = np.concatenate([exp_feat, cmask_feat, gamma_feat], axis=-1)
    emb = np.concatenate([emb, np.sign(dist)[:, None] * emb], axis=-1)
    _POS_CACHE["emb"] = emb  # (4095, 1536) float64
    return emb


# ----------------------------------------------------------------------------
# device graph (identical for all cores; per-core data differs)
_GRAPH_CACHE = {}


def _build_graph():
    if "nc" in _GRAPH_CACHE:
        return _GRAPH_CACHE["nc"]
    nc = bass.Bass()

    xT = nc.declare_dram_parameter("xT", [DM, N], FP16, isOutput=False)
    wq = nc.declare_dram_parameter("wq", [DM, HD], FP16, isOutput=False)
    wkv = nc.declare_dram_parameter("wkv", [DM, 2 * HD], FP16, isOutput=False)
    ptab = nc.declare_dram_parameter("ptab", [HD, 2 * N], FP16, isOutput=False)
    cbias = nc.declare_dram_parameter("cbias", [HD, 1], F32, isOutput=False)
    pbias = nc.declare_dram_parameter("pbias", [HD, 1], F32, isOutput=False)
    wout = nc.declare_dram_parameter("wout", [HD, DM], BF16, isOutput=False)
    out_ext = nc.declare_dram_parameter("out", [N, DM], BF16, isOutput=True)
    rs_ext = nc.declare_dram_parameter("rs", [1, N], F32, isOutput=True)
    debug = bool(os.environ.get("KERNEL_DEBUG"))
    if debug:
        dbg_t0 = nc.declare_dram_parameter("dbg_t0", [512, TSTRIDE], FP16,
                                           isOutput=True)
        dbg_anT = nc.declare_dram_parameter("dbg_anT", [128, KT * 512], FP16,
                                            isOutput=True)
        dbg_anF = nc.declare_dram_parameter("dbg_anF", [128, KT * 512], BF16,
                                            isOutput=True)

    # internal DRAM: one T scratch per q-group of 512 rows (raw fp16 scores).
    # Tile a (local row r, window col c in [0, WIN)) writes flat offset
    #   128*al*(TSTRIDE-1) + r*TSTRIDE + c      (al = a % 4, local tile in grp)
    # so the group read for (kp, kt, q) is the clean 2D shear
    #   127 + q*(TSTRIDE-1) + 128*kt + kp       (q local in [0, 512)).
    t_dram = [nc.dram_tensor(f"tscr{g}", [512, TSTRIDE], FP16) for g in range(NG)]

    Act = mybir.ActivationFunctionType

    with TileContext(nc) as tc:
        with contextlib.ExitStack() as ctx:
            persist = ctx.enter_context(tc.tile_pool(name="persist", bufs=1))
            work = ctx.enter_context(tc.tile_pool(name="work", bufs=2))

            qcT = persist.tile([HD, N], FP16, tag="qcT")
            qpT = persist.tile([HD, N], FP16, tag="qpT")
            kvT = persist.tile([128, N], FP16, tag="kvT")   # k rows 0:64, v rows 64:128
            PT = persist.tile([HD, 2 * N], FP16, tag="PT")
            vsb = [persist.tile([128, HD + 1], BF16, tag=f"v{k}", name=f"v{k}")
                   for k in range(KT)]
            ident = persist.tile([128, 128], FP16, tag="ident")
            make_identity(nc, ident)
            rs_sb = persist.tile([1, N], F32, tag="rs_sb")
            wout_sb = persist.tile([HD, DM], BF16, tag="wout")
            cb_sb = persist.tile([HD, 1], F32, tag="cb")
            pb_sb = persist.tile([HD, 1], F32, tag="pb")

            anT = [None] * NG
            anF = [None] * NG
            og = {}

            def alloc_an(g):
                anT[g] = work.tile([128, KT, 512], FP16, tag="anT", bufs=4,
                                   name=f"anT{g}")
                anF[g] = work.tile([128, KT, 512], BF16, tag="anF", bufs=2,
                                   name=f"anF{g}")

            def produce(a, tpool, ttag, tbufs):
                """T matmuls -> fp16 tsb (raw scores) -> DRAM write."""
                g, al = a // 4, a % 4
                w0 = 1920 - 128 * a
                tsb = work.tile([128, WIN], FP16, tag="tsb", bufs=3,
                                name=f"tsb{a}")
                for ci, (off, w) in enumerate(CHUNKS):
                    tp = tpool.tile([128, 512], F32, tag=ttag, bufs=tbufs,
                                    name=f"tp{a}_{off}")
                    nc.tensor.matmul(tp[:, :w], qpT[:, 128 * a:128 * (a + 1)],
                                     PT[:, w0 + off:w0 + off + w],
                                     start=True, stop=True)
                    if ci in (0, 2):
                        nc.scalar.copy(tsb[:, off:off + w], tp[:, :w])
                    else:
                        nc.vector.tensor_copy(tsb[:, off:off + w], tp[:, :w])
                dst = bass.AP(tensor=t_dram[g].ap().tensor,
                              offset=128 * al * (TSTRIDE - 1),
                              ap=[[TSTRIDE, 128], [1, WIN]])
                eng = nc.sync if a % 2 == 0 else nc.scalar
                eng.dma_start(out=dst, in_=tsb[:, 0:WIN])

            def shear_half(g, h):
                """fused shear+transpose DGE read of half-group (g, h):
                rows 256h..256h+256 of group g -> [kp 128, kt 16, q 256]."""
                src = bass.AP(tensor=t_dram[g].ap().tensor,
                              offset=127 + 256 * h * (TSTRIDE - 1),
                              ap=[[TSTRIDE - 1, 256], [1, N]])
                nc.sync.dma_start_transpose(anT[g][:, :, 256 * h:256 * (h + 1)],
                                            src)

            # ---------------- phase 1: projections + early produce ----------
            with contextlib.ExitStack() as ph1:
                wpool = ph1.enter_context(tc.tile_pool(name="wpool", bufs=1))
                xstream = ph1.enter_context(tc.tile_pool(name="xstream", bufs=4))
                ppsum = ph1.enter_context(
                    tc.tile_pool(name="ppsum", bufs=1, space="PSUM"))

                wq_sb, wkv_sb = [], []
                for f in range(12):
                    t = wpool.tile([128, HD], FP16, tag=f"wq{f}", name=f"wq{f}")
                    wq_sb.append(t)
                    t = wpool.tile([128, 2 * HD], FP16, tag=f"wkv{f}", name=f"wkv{f}")
                    wkv_sb.append(t)
                nc.sync.dma_start(out=wq_sb[0], in_=wq[0:128, :])
                nc.scalar.dma_start(out=wkv_sb[0], in_=wkv[0:128, :])
                # first x tile in 4 separate column tiles so matmul (f0, i)
                # can start as soon as chunk i lands
                xt0c = []
                for i in range(4):
                    t = xstream.tile([128, 512], FP16, tag=f"xt0c{i}",
                                     name=f"xt0c{i}")
                    eng = nc.scalar if i % 2 == 0 else nc.sync
                    eng.dma_start(out=t, in_=xT[0:128, 512 * i:512 * (i + 1)])
                    xt0c.append(t)
                nc.gpsimd.dma_start(out=cb_sb, in_=cbias[:, :])
                nc.gpsimd.dma_start(out=pb_sb, in_=pbias[:, :])
                # weight tiles go via the software DGE so the two hardware
                # queues carry nothing but the x stream
                for f in range(1, 12):
                    nc.gpsimd.dma_start(out=wq_sb[f],
                                        in_=wq[128 * f:128 * (f + 1), :])
                    nc.gpsimd.dma_start(out=wkv_sb[f],
                                        in_=wkv[128 * f:128 * (f + 1), :])
                nc.gpsimd.dma_start(out=PT, in_=ptab[:, :])
                nc.gpsimd.dma_start(out=wout_sb, in_=wout[:, :])

                q_ps = [ppsum.tile([HD, 512], F32, tag="pq", bufs=4,
                                   name=f"qps{i}") for i in range(4)]
                kv_ps = [ppsum.tile([128, 512], F32, tag="pkv", bufs=4,
                                    name=f"kvps{i}") for i in range(4)]
                for f in range(12):
                    if f != 0:
                        xt = xstream.tile([128, N], FP16, tag="xt")
                        eng = nc.sync if f % 2 == 0 else nc.scalar
                        eng.dma_start(out=xt, in_=xT[128 * f:128 * (f + 1), :])
                    for i in range(4):
                        rhs = xt0c[i] if f == 0 else xt[:, 512 * i:512 * (i + 1)]
                        nc.tensor.matmul(q_ps[i], wq_sb[f], rhs,
                                         start=(f == 0), stop=(f == 11))
                        nc.tensor.matmul(kv_ps[i], wkv_sb[f], rhs,
                                         start=(f == 0), stop=(f == 11))

                # ones column helper: f(0*x + 1) = 1 via ACT immediate
                onescol = persist.tile([128, 1], BF16, tag="ones")
                nc.scalar.activation(onescol, ident[:, 0:1],
                                     Act.Identity, scale=0.0, bias=1.0)

                # copy-out: qpT first (unblocks T-matmul pipeline), then qcT/kv
                for i in range(4):
                    nc.scalar.activation(qpT[:, 512 * i:512 * (i + 1)], q_ps[i],
                                         Act.Identity, bias=pb_sb)
                for i in range(4):
                    nc.scalar.activation(qcT[:, 512 * i:512 * (i + 1)], q_ps[i],
                                         Act.Identity, bias=cb_sb)
                    nc.vector.tensor_copy(kvT[:, 512 * i:512 * (i + 1)], kv_ps[i])
                # v transpose to seq-major bf16 (+ ones column for row sums)
                for k in range(KT):
                    tp = ppsum.tile([128, HD], FP16, tag="pq", bufs=4)
                    nc.tensor.transpose(tp, kvT[HD:128, 128 * k:128 * (k + 1)],
                                        ident[HD:128, HD:128])
                    nc.vector.tensor_copy(vsb[k][:, 0:HD], tp)
                    nc.vector.tensor_copy(vsb[k][:, HD:HD + 1], onescol)

            # ---------------- phase 2: pipelined scores/softmax/av/fin -------
            psum = ctx.enter_context(tc.tile_pool(name="psum", bufs=1, space="PSUM"))

            def consume(g):
                """scores^T = K Qc^T (psum) + anT (DVE), exp (ACT) -> anF."""
                for kt in range(KT):
                    sp = psum.tile([128, 512], F32, tag="sc", bufs=3,
                                   name=f"sp{g}_{kt}")
                    nc.tensor.matmul(sp, kvT[0:HD, 128 * kt:128 * (kt + 1)],
                                     qcT[:, 512 * g:512 * (g + 1)],
                                     start=True, stop=True)
                    nc.vector.tensor_add(sp, sp, anT[g][:, kt, :])
                    nc.scalar.activation(anF[g][:, kt, :], sp, Act.Exp)

            def av(g):
                """attn^T stationary V: otp[hd|ones, q 512] accum over kt.
                Two independent 256-wide accumulation chains interleave so
                consecutive matmuls never wait on the same psum region."""
                otp = psum.tile([HD + 1, 512], F32, tag="av", bufs=1,
                                name=f"otp{g}")
                for kt in range(KT):
                    for h in range(2):
                        nc.tensor.matmul(otp[:, 256 * h:256 * (h + 1)],
                                         vsb[kt],
                                         anF[g][:, kt, 256 * h:256 * (h + 1)],
                                         start=(kt == 0), stop=(kt == KT - 1))
                og[g] = work.tile([HD, 512], BF16, tag="og", bufs=2,
                                  name=f"og{g}")
                nc.scalar.activation(og[g], otp[0:HD, :], Act.Copy)
                nc.scalar.copy(rs_sb[:, 512 * g:512 * (g + 1)],
                               otp[HD:HD + 1, :])

            def fin(a):
                """partial out rows for q-tile a: og_g[:, sub]^T @ wout."""
                g, b = a // 4, a % 4
                fo = work.tile([128, DM], BF16, tag="fo", bufs=3,
                               name=f"fo{a}")
                for j in range(3):
                    fp = psum.tile([128, 512], F32, tag="fin", bufs=2,
                                   name=f"fp{a}_{j}")
                    nc.tensor.matmul(fp, og[g][:, 128 * b:128 * (b + 1)],
                                     wout_sb[:, 512 * j:512 * (j + 1)],
                                     start=True, stop=True)
                    nc.vector.tensor_copy(fo[:, 512 * j:512 * (j + 1)], fp)
                eng = nc.scalar if a % 2 == 0 else nc.sync
                eng.dma_start(out=out_ext[128 * a:128 * (a + 1), :], in_=fo)

            for a in range(0, 8):
                produce(a, psum, "T", 2)
            alloc_an(0)
            shear_half(0, 0)
            shear_half(0, 1)
            alloc_an(1)
            shear_half(1, 0)
            shear_half(1, 1)
            consume(0)
            if debug:
                nc.sync.dma_start(out=dbg_t0[:, :], in_=t_dram[0][:, :])
                nc.sync.dma_start(out=dbg_anT[:, :],
                                  in_=anT[0].rearrange("p a b -> p (a b)"))
            av(0)
            if debug:
                nc.sync.dma_start(out=dbg_anF[:, :],
                                  in_=anF[0].rearrange("p a b -> p (a b)"))
            for a in range(8, 12):
                produce(a, psum, "T", 2)
            alloc_an(2)
            shear_half(2, 0)
            shear_half(2, 1)
            consume(1)
            av(1)
            for a in range(12, 16):
                produce(a, psum, "T", 2)
            alloc_an(3)
            shear_half(3, 0)
            shear_half(3, 1)
            for a in range(0, 4):
                fin(a)
            consume(2)
            av(2)
            for a in range(4, 12):
                fin(a)
            consume(3)
            av(3)
            for a in range(12, 16):
                fin(a)
            nc.sync.dma_start(out=rs_ext[:, :], in_=rs_sb)

    # wait-split post-processing hook
    orig = nc.to_json_bytes
    nc.to_json_bytes = lambda: _split_waits(orig())
    _GRAPH_CACHE["nc"] = nc
    return nc


# ----------------------------------------------------------------------------
def _prep_inputs(x, Wq, Wk, Wv, content_bias, pos_bias, Wp_w, Wp_b, Wout_w, Wout_b):
    x = np.ascontiguousarray(np.asarray(x, dtype=np.float32))
    Wq = np.asarray(Wq, np.float32); Wk = np.asarray(Wk, np.float32)
    Wv = np.asarray(Wv, np.float32)
    content_bias = np.asarray(content_bias, np.float32)
    pos_bias = np.asarray(pos_bias, np.float32)
    Wp_w = np.asarray(Wp_w, np.float32); Wp_b = np.asarray(Wp_b, np.float32)
    Wout_w = np.asarray(Wout_w, np.float32)

    scale = HD ** -0.5
    xT = np.ascontiguousarray(x[0].T)                    # (1536, 2048)
    emb = _pos_embed()                                   # (4095, 1536) f64
    wp_sum = Wp_w.reshape(DM, H, HD).sum(axis=1)         # (1536, 64)
    wp_b_sum = Wp_b.reshape(H, HD).sum(axis=0)           # (64,)
    # positional table: P = emb @ wp_sum + b  (weight preprocessing)
    P = emb @ wp_sum.astype(np.float64) + wp_b_sum.astype(np.float64)
    PTh = np.zeros((HD, 2 * N), np.float16)
    PTh[:, :2 * N - 1] = P.T.astype(np.float16)          # (64, 4096)
    xT16 = xT.astype(np.float16)

    in_maps = []
    for c in range(NCORES):
        sl = slice(HD * c, HD * (c + 1))
        in_maps.append({
            "xT": xT16,
            "wq": np.ascontiguousarray(Wq[:, sl] * scale).astype(np.float16),
            "wkv": np.ascontiguousarray(
                np.concatenate([Wk[:, sl], Wv[:, sl]], axis=1)).astype(np.float16),
            "ptab": PTh,
            "cbias": np.ascontiguousarray(content_bias[c, 0, :, None]),
            "pbias": np.ascontiguousarray(pos_bias[c, 0, :, None]),
            "wout": np.ascontiguousarray(
                Wout_w[sl, :]).astype(ml_dtypes.bfloat16),
        })
    return in_maps


def kernel(x, Wq, Wk, Wv, content_bias, pos_bias, Wp_w, Wp_b, Wout_w, Wout_b):
    global _LAST_RESULT
    in_maps = _prep_inputs(x, Wq, Wk, Wv, content_bias, pos_bias,
                           Wp_w, Wp_b, Wout_w, Wout_b)
    nc = _build_graph()
    trace = bool(os.environ.get("KERNEL_TRACE"))
    res = run_bass_kernel_spmd(nc, in_maps, core_ids=list(range(NCORES)),
                               trace=trace, trace_cores=[0] if trace else None)
    _LAST_RESULT = res
    out = np.zeros((N, DM), np.float64)
    for c in range(NCORES):
        part = np.asarray(res.results[c]["out"], dtype=np.float64)  # (N, DM)
        rs = np.asarray(res.results[c]["rs"], dtype=np.float64)     # (1, N)
        out += part / rs[0][:, None]
    out += np.asarray(np.asarray(Wout_b, np.float32), np.float64)[None, :]
    return out[None].astype(np.float32)


# revision 37
# speedup vs baseline: 1.0149x; 1.0149x over previous
"""Trainium2 Bass kernel for nn_Attention_63745904608049.

Relative-position attention (TransformerXL-style shift, Enformer-style pos
basis), batch 1, seq 2048, d_model 1536, 8 heads x 64. Head-parallel over 8
NeuronCores.

v4c: [k, q]-oriented score pipeline. The relative shift AND the transpose
needed by the attn*V matmul are fused into ONE DGE transposed read of the
positional-score scratch in DRAM (dma_start_transpose from a sheared flat
AP). Content scores are computed directly transposed (K^T tiles stationary).
Softmax combine is multiplicative: anF = exp(T)^T * exp(C^T), with exp(T)
taken in bulk 2048-wide SBUF ops and exp(C) fused into the psum evacuation.
attn*V consumes the combined [k, q] tiles directly; softmax row sums ride a
ones-column in V. Host divides by row sums, sums partials over cores, adds
the output bias. No device collectives.

Self-contained: hardcodes shapes, builds one SPMD Bass graph, runs it on
cores 0-7 via run_bass_kernel_spmd, and reassembles the full output.
"""
import contextlib
import ctypes
import math
import os
import sys
import types

import numpy as np
import ml_dtypes

import concourse.bass as bass
import concourse.mybir as mybir
from concourse.tile import TileContext
from concourse.masks import make_identity
from concourse.bass_utils import run_bass_kernel_spmd

# ----------------------------------------------------------------------------
# problem constants
N = 2048
DM = 1536
H = 8
HD = 64
INNER = H * HD            # 512
NCORES = 8
QT = N // 128             # 16 query tiles
KT = N // 128             # 16 key tiles
NG = 4                    # 4 query groups of 512
WIN = 2175                # per-q-tile pos table window (128 + 2048 - 1)
TSTRIDE = 2304            # padded row stride of the T scratch (elements)
CHUNKS = [(0, 512), (512, 512), (1024, 512), (1536, 512), (2048, 127)]
F32 = mybir.dt.float32
BF16 = mybir.dt.bfloat16
FP16 = mybir.dt.float16

_LAST_RESULT = None       # BassKernelResults of the last run (for test.py)


# ----------------------------------------------------------------------------
# axon NTFF profiling hook (lets BASS_TRACE=1 produce exec_time_ns under axon)
def _install_ntff_hook(so_path="/opt/axon/libaxon_pjrt.so"):
    try:
        import antenv.axon_hooks  # noqa: F401
        return
    except ImportError:
        pass
    try:
        lib = ctypes.CDLL(so_path)
    except OSError:
        return
    if not hasattr(lib, "axon_start_nrt_profile"):
        return
    lib.axon_start_nrt_profile.argtypes = [ctypes.POINTER(ctypes.c_int64), ctypes.c_size_t]
    lib.axon_start_nrt_profile.restype = ctypes.c_int64
    lib.axon_stop_nrt_profile.argtypes = [ctypes.c_char_p]
    lib.axon_stop_nrt_profile.restype = ctypes.c_int64

    @contextlib.contextmanager
    def _hook(output_dir, device_ids):
        import jax
        jax.devices()
        if device_ids:
            ids = (ctypes.c_int64 * len(device_ids))(*device_ids)
            rc = lib.axon_start_nrt_profile(ids, len(device_ids))
        else:
            rc = lib.axon_start_nrt_profile(None, 0)
        if rc != 0:
            raise RuntimeError(f"axon_start_nrt_profile rc={rc}")
        try:
            yield
        finally:
            n = lib.axon_stop_nrt_profile(str(output_dir).encode())
            print(f"ntff profile: {n} file(s) written to {output_dir}")

    mod = types.ModuleType("antenv.axon_hooks")
    mod.get_axon_ntff_profile_hook = lambda: _hook
    mod.set_axon_ntff_profile_hook = lambda h: None
    sys.modules["antenv.axon_hooks"] = mod


_install_ntff_hook()


# ----------------------------------------------------------------------------
# BIR post-processing: this container's walrus build rejects instructions with
# more than one sync wait; split extra waits onto preceding NoOps.
def _split_waits(bir_bytes, maxw=1):
    import json
    d = json.loads(bir_bytes)
    counter = [0]
    for fn in d["functions"]:
        for blk in fn["blocks"]:
            out = []
            for ins in blk["instructions"]:
                si = ins.get("sync_info")
                waits = (si or {}).get("on_wait") or []
                if len(waits) > maxw:
                    excess = waits[:-maxw]
                    ins["sync_info"]["on_wait"] = waits[-maxw:]
                    for i in range(0, len(excess), maxw):
                        counter[0] += 1
                        nop = {
                            "engine": ins["engine"],
                            "ins": [],
                            "outs": [],
                            "name": f"I-waitsplit-{counter[0]}",
                            "opcode": "NoOp",
                            "sync_info": {"on_update": [],
                                          "on_wait": excess[i:i + maxw]},
                        }
                        if "debug" in ins:
                            nop["debug"] = ins["debug"]
                        out.append(nop)
                out.append(ins)
            blk["instructions"] = out
    return json.dumps(d).encode()


# ----------------------------------------------------------------------------
# host-side positional embedding table (pure function of N, DM)
_POS_CACHE = {}


def _pos_embed():
    if "emb" in _POS_CACHE:
        return _POS_CACHE["emb"]
    n, fs = N, DM
    nb = fs // 6  # 256
    dist = np.arange(-n + 1, n, dtype=np.float64)
    adist = np.abs(dist)[:, None]

    max_range = math.log(n) / math.log(2.0)
    half_life = 2.0 ** np.linspace(3.0, max_range, nb)
    exp_feat = np.exp(-math.log(2.0) / half_life[None, :] * adist)

    with np.errstate(over="ignore"):
        center_widths = 2.0 ** np.arange(1, nb + 1, dtype=np.float64) - 1.0
    cmask_feat = (center_widths[None, :] > adist).astype(np.float64)

    stddev = n / (2.0 * nb)
    start_mean = n / nb
    mean = np.linspace(start_mean, float(n), nb)[None, :]
    conc = (mean / stddev) ** 2
    rate = mean / stddev ** 2
    with np.errstate(divide="ignore", invalid="ignore"):
        log_unnorm = (conc - 1.0) * np.log(adist) - rate * adist
    lgamma = np.vectorize(math.lgamma)
    log_norm = lgamma(conc) - conc * np.log(rate)
    with np.errstate(invalid="ignore"):
        prob = np.exp(log_unnorm - log_norm) + 1e-08
    prob = np.nan_to_num(prob, nan=1e-08)  # adist=0: 0*inf -> use limit 0, then +eps
    zrow = np.where(adist[:, 0] == 0)[0]
    prob[zrow, :] = 1e-08
    gamma_feat = prob / prob.max(axis=-1, keepdims=True)

    emb = np.concatenate([exp_feat, cmask_feat, gamma_feat], axis=-1)
    emb = np.concatenate([emb, np.sign(dist)[:, None] * emb], axis=-1)
    _POS_CACHE["emb"] = emb  # (4095, 1536) float64
    return emb


# ----------------------------------------------------------------------------
# device graph (identical for all cores; per-core data differs)
_GRAPH_CACHE = {}


def _build_graph():
    if "nc" in _GRAPH_CACHE:
        return _GRAPH_CACHE["nc"]
    nc = bass.Bass()

    xT = nc.declare_dram_parameter("xT", [DM, N], FP16, isOutput=False)
    wq = nc.declare_dram_parameter("wq", [DM, HD], FP16, isOutput=False)
    wkv = nc.declare_dram_parameter("wkv", [DM, 2 * HD], FP16, isOutput=False)
    ptab = nc.declare_dram_parameter("ptab", [HD, 2 * N], FP16, isOutput=False)
    cbias = nc.declare_dram_parameter("cbias", [HD, 1], F32, isOutput=False)
    pbias = nc.declare_dram_parameter("pbias", [HD, 1], F32, isOutput=False)
    wout = nc.declare_dram_parameter("wout", [HD, DM], BF16, isOutput=False)
    out_ext = nc.declare_dram_parameter("out", [N, DM], BF16, isOutput=True)
    rs_ext = nc.declare_dram_parameter("rs", [1, N], F32, isOutput=True)
    debug = bool(os.environ.get("KERNEL_DEBUG"))
    if debug:
        dbg_t0 = nc.declare_dram_parameter("dbg_t0", [512, TSTRIDE], FP16,
                                           isOutput=True)
        dbg_anT = nc.declare_dram_parameter("dbg_anT", [128, KT * 512], FP16,
                                            isOutput=True)
        dbg_anF = nc.declare_dram_parameter("dbg_anF", [128, KT * 512], BF16,
                                            isOutput=True)

    # internal DRAM: one T scratch per q-group of 512 rows (raw fp16 scores).
    # Tile a (local row r, window col c in [0, WIN)) writes flat offset
    #   128*al*(TSTRIDE-1) + r*TSTRIDE + c      (al = a % 4, local tile in grp)
    # so the group read for (kp, kt, q) is the clean 2D shear
    #   127 + q*(TSTRIDE-1) + 128*kt + kp       (q local in [0, 512)).
    t_dram = [nc.dram_tensor(f"tscr{g}", [512, TSTRIDE], FP16) for g in range(NG)]

    Act = mybir.ActivationFunctionType

    with TileContext(nc) as tc:
        with contextlib.ExitStack() as ctx:
            persist = ctx.enter_context(tc.tile_pool(name="persist", bufs=1))
            work = ctx.enter_context(tc.tile_pool(name="work", bufs=2))

            qcT = persist.tile([HD, N], FP16, tag="qcT")
            qpT = persist.tile([HD, N], FP16, tag="qpT")
            kvT = persist.tile([128, N], FP16, tag="kvT")   # k rows 0:64, v rows 64:128
            PT = persist.tile([HD, 2 * N], FP16, tag="PT")
            vsb = [persist.tile([128, HD + 1], BF16, tag=f"v{k}", name=f"v{k}")
                   for k in range(KT)]
            ident = persist.tile([128, 128], FP16, tag="ident")
            make_identity(nc, ident)
            rs_sb = persist.tile([1, N], F32, tag="rs_sb")
            wout_sb = persist.tile([HD, DM], BF16, tag="wout")
            cb_sb = persist.tile([HD, 1], F32, tag="cb")
            pb_sb = persist.tile([HD, 1], F32, tag="pb")

            anT = [None] * NG
            anF = [None] * NG
            og = {}

            def alloc_an(g):
                anT[g] = work.tile([128, KT, 512], FP16, tag="anT", bufs=4,
                                   name=f"anT{g}")
                anF[g] = work.tile([128, KT, 512], BF16, tag="anF", bufs=2,
                                   name=f"anF{g}")

            def produce(a, tpool, ttag, tbufs):
                """T matmuls -> fp16 tsb (raw scores) -> DRAM write."""
                g, al = a // 4, a % 4
                w0 = 1920 - 128 * a
                tsb = work.tile([128, WIN], FP16, tag="tsb", bufs=3,
                                name=f"tsb{a}")
                for ci, (off, w) in enumerate(CHUNKS):
                    tp = tpool.tile([128, 512], F32, tag=ttag, bufs=tbufs,
                                    name=f"tp{a}_{off}")
                    nc.tensor.matmul(tp[:, :w], qpT[:, 128 * a:128 * (a + 1)],
                                     PT[:, w0 + off:w0 + off + w],
                                     start=True, stop=True)
                    if ci in (0, 2):
                        nc.scalar.copy(tsb[:, off:off + w], tp[:, :w])
                    else:
                        nc.vector.tensor_copy(tsb[:, off:off + w], tp[:, :w])
                dst = bass.AP(tensor=t_dram[g].ap().tensor,
                              offset=128 * al * (TSTRIDE - 1),
                              ap=[[TSTRIDE, 128], [1, WIN]])
                eng = nc.sync if a % 2 == 0 else nc.scalar
                eng.dma_start(out=dst, in_=tsb[:, 0:WIN])

            def shear_half(g, h):
                """fused shear+transpose DGE read of half-group (g, h):
                rows 256h..256h+256 of group g -> [kp 128, kt 16, q 256]."""
                src = bass.AP(tensor=t_dram[g].ap().tensor,
                              offset=127 + 256 * h * (TSTRIDE - 1),
                              ap=[[TSTRIDE - 1, 256], [1, N]])
                nc.sync.dma_start_transpose(anT[g][:, :, 256 * h:256 * (h + 1)],
                                            src)

            # ---------------- phase 1: projections + early produce ----------
            with contextlib.ExitStack() as ph1:
                wpool = ph1.enter_context(tc.tile_pool(name="wpool", bufs=1))
                xstream = ph1.enter_context(tc.tile_pool(name="xstream", bufs=4))
                ppsum = ph1.enter_context(
                    tc.tile_pool(name="ppsum", bufs=1, space="PSUM"))

                wq_sb, wkv_sb = [], []
                for f in range(12):
                    t = wpool.tile([128, HD], FP16, tag=f"wq{f}", name=f"wq{f}")
                    wq_sb.append(t)
                    t = wpool.tile([128, 2 * HD], FP16, tag=f"wkv{f}", name=f"wkv{f}")
                    wkv_sb.append(t)
                nc.sync.dma_start(out=wq_sb[0], in_=wq[0:128, :])
                nc.scalar.dma_start(out=wkv_sb[0], in_=wkv[0:128, :])
                # first x tile in 4 separate column tiles so matmul (f0, i)
                # can start as soon as chunk i lands
                xt0c = []
                for i in range(4):
                    t = xstream.tile([128, 512], FP16, tag=f"xt0c{i}",
                                     name=f"xt0c{i}")
                    eng = nc.scalar if i % 2 == 0 else nc.sync
                    eng.dma_start(out=t, in_=xT[0:128, 512 * i:512 * (i + 1)])
                    xt0c.append(t)
                nc.gpsimd.dma_start(out=cb_sb, in_=cbias[:, :])
                nc.gpsimd.dma_start(out=pb_sb, in_=pbias[:, :])
                # weight tiles go via the software DGE so the two hardware
                # queues carry nothing but the x stream
                for f in range(1, 12):
                    nc.gpsimd.dma_start(out=wq_sb[f],
                                        in_=wq[128 * f:128 * (f + 1), :])
                    nc.gpsimd.dma_start(out=wkv_sb[f],
                                        in_=wkv[128 * f:128 * (f + 1), :])
                nc.gpsimd.dma_start(out=PT, in_=ptab[:, :])
                nc.gpsimd.dma_start(out=wout_sb, in_=wout[:, :])

                q_ps = [ppsum.tile([HD, 512], F32, tag="pq", bufs=4,
                                   name=f"qps{i}") for i in range(4)]
                kv_ps = [ppsum.tile([128, 512], F32, tag="pkv", bufs=4,
                                    name=f"kvps{i}") for i in range(4)]
                for f in range(12):
                    if f != 0:
                        xt = xstream.tile([128, N], FP16, tag="xt")
                        eng = nc.sync if f % 2 == 0 else nc.scalar
                        eng.dma_start(out=xt, in_=xT[128 * f:128 * (f + 1), :])
                    for i in range(4):
                        rhs = xt0c[i] if f == 0 else xt[:, 512 * i:512 * (i + 1)]
                        nc.tensor.matmul(q_ps[i], wq_sb[f], rhs,
                                         start=(f == 0), stop=(f == 11))
                        nc.tensor.matmul(kv_ps[i], wkv_sb[f], rhs,
                                         start=(f == 0), stop=(f == 11))

                # ones column helper: f(0*x + 1) = 1 via ACT immediate
                onescol = persist.tile([128, 1], BF16, tag="ones")
                nc.scalar.activation(onescol, ident[:, 0:1],
                                     Act.Identity, scale=0.0, bias=1.0)

                # copy-out: qpT first (unblocks T-matmul pipeline), then qcT/kv
                for i in range(4):
                    nc.scalar.activation(qpT[:, 512 * i:512 * (i + 1)], q_ps[i],
                                         Act.Identity, bias=pb_sb)
                for i in range(4):
                    nc.scalar.activation(qcT[:, 512 * i:512 * (i + 1)], q_ps[i],
                                         Act.Identity, bias=cb_sb)
                    nc.vector.tensor_copy(kvT[:, 512 * i:512 * (i + 1)], kv_ps[i])
                # v transpose to seq-major bf16 (+ ones column for row sums)
                for k in range(KT):
                    tp = ppsum.tile([128, HD], FP16, tag="pq", bufs=4)
                    nc.tensor.transpose(tp, kvT[HD:128, 128 * k:128 * (k + 1)],
                                        ident[HD:128, HD:128])
                    nc.vector.tensor_copy(vsb[k][:, 0:HD], tp)
                    nc.vector.tensor_copy(vsb[k][:, HD:HD + 1], onescol)

            # ---------------- phase 2: pipelined scores/softmax/av/fin -------
            psum = ctx.enter_context(tc.tile_pool(name="psum", bufs=1, space="PSUM"))

            def consume(g):
                """scores^T = K Qc^T (psum) + anT (DVE), exp (ACT) -> anF."""
                for kt in range(KT):
                    sp = psum.tile([128, 512], F32, tag="sc", bufs=3,
                                   name=f"sp{g}_{kt}")
                    nc.tensor.matmul(sp, kvT[0:HD, 128 * kt:128 * (kt + 1)],
                                     qcT[:, 512 * g:512 * (g + 1)],
                                     start=True, stop=True)
                    nc.vector.tensor_add(sp, sp, anT[g][:, kt, :])
                    nc.scalar.activation(anF[g][:, kt, :], sp, Act.Exp)

            def av(g):
                """attn^T stationary V: two psum accumulators (even/odd kt)
                so consecutive matmuls pipeline instead of waiting on the
                same accumulation chain; DVE merges them on evacuation."""
                otpA = psum.tile([HD + 1, 512], F32, tag="av", bufs=2,
                                 name=f"otpA{g}")
                otpB = psum.tile([HD + 1, 512], F32, tag="av", bufs=2,
                                 name=f"otpB{g}")
                for kt in range(KT):
                    dst = otpA if kt % 2 == 0 else otpB
                    nc.tensor.matmul(dst, vsb[kt], anF[g][:, kt, :],
                                     start=(kt < 2), stop=(kt >= KT - 2))
                og[g] = work.tile([HD, 512], BF16, tag="og", bufs=2,
                                  name=f"og{g}")
                ogB = work.tile([HD + 1, 512], F32, tag="ogB", bufs=2,
                                name=f"ogB{g}")
                nc.scalar.copy(ogB, otpB)
                nc.vector.tensor_add(og[g], otpA[0:HD, :], ogB[0:HD, :])
                nc.vector.tensor_add(rs_sb[:, 512 * g:512 * (g + 1)],
                                     otpA[HD:HD + 1, :], ogB[HD:HD + 1, :])

            def fin(a):
                """partial out rows for q-tile a: og_g[:, sub]^T @ wout."""
                g, b = a // 4, a % 4
                fo = work.tile([128, DM], BF16, tag="fo", bufs=3,
                               name=f"fo{a}")
                for j in range(3):
                    fp = psum.tile([128, 512], F32, tag="fin", bufs=1,
                                   name=f"fp{a}_{j}")
                    nc.tensor.matmul(fp, og[g][:, 128 * b:128 * (b + 1)],
                                     wout_sb[:, 512 * j:512 * (j + 1)],
                                     start=True, stop=True)
                    nc.vector.tensor_copy(fo[:, 512 * j:512 * (j + 1)], fp)
                eng = nc.scalar if a % 2 == 0 else nc.sync
                eng.dma_start(out=out_ext[128 * a:128 * (a + 1), :], in_=fo)

            for a in range(0, 8):
                produce(a, psum, "T", 2)
            alloc_an(0)
            shear_half(0, 0)
            shear_half(0, 1)
            alloc_an(1)
            shear_half(1, 0)
            shear_half(1, 1)
            consume(0)
            if debug:
                nc.sync.dma_start(out=dbg_t0[:, :], in_=t_dram[0][:, :])
                nc.sync.dma_start(out=dbg_anT[:, :],
                                  in_=anT[0].rearrange("p a b -> p (a b)"))
            av(0)
            if debug:
                nc.sync.dma_start(out=dbg_anF[:, :],
                                  in_=anF[0].rearrange("p a b -> p (a b)"))
            for a in range(8, 12):
                produce(a, psum, "T", 2)
            alloc_an(2)
            shear_half(2, 0)
            shear_half(2, 1)
            consume(1)
            av(1)
            for a in range(12, 16):
                produce(a, psum, "T", 2)
            alloc_an(3)
            shear_half(3, 0)
            shear_half(3, 1)
            for a in range(0, 4):
                fin(a)
            consume(2)
            av(2)
            for a in range(4, 12):
                fin(a)
            consume(3)
            av(3)
            for a in range(12, 16):
                fin(a)
            nc.sync.dma_start(out=rs_ext[:, :], in_=rs_sb)

    # wait-split post-processing hook
    orig = nc.to_json_bytes
    nc.to_json_bytes = lambda: _split_waits(orig())
    _GRAPH_CACHE["nc"] = nc
    return nc


# ----------------------------------------------------------------------------
def _prep_inputs(x, Wq, Wk, Wv, content_bias, pos_bias, Wp_w, Wp_b, Wout_w, Wout_b):
    x = np.ascontiguousarray(np.asarray(x, dtype=np.float32))
    Wq = np.asarray(Wq, np.float32); Wk = np.asarray(Wk, np.float32)
    Wv = np.asarray(Wv, np.float32)
    content_bias = np.asarray(content_bias, np.float32)
    pos_bias = np.asarray(pos_bias, np.float32)
    Wp_w = np.asarray(Wp_w, np.float32); Wp_b = np.asarray(Wp_b, np.float32)
    Wout_w = np.asarray(Wout_w, np.float32)

    scale = HD ** -0.5
    xT = np.ascontiguousarray(x[0].T)                    # (1536, 2048)
    emb = _pos_embed()                                   # (4095, 1536) f64
    wp_sum = Wp_w.reshape(DM, H, HD).sum(axis=1)         # (1536, 64)
    wp_b_sum = Wp_b.reshape(H, HD).sum(axis=0)           # (64,)
    # positional table: P = emb @ wp_sum + b  (weight preprocessing)
    P = emb @ wp_sum.astype(np.float64) + wp_b_sum.astype(np.float64)
    PTh = np.zeros((HD, 2 * N), np.float16)
    PTh[:, :2 * N - 1] = P.T.astype(np.float16)          # (64, 4096)
    xT16 = xT.astype(np.float16)

    in_maps = []
    for c in range(NCORES):
        sl = slice(HD * c, HD * (c + 1))
        in_maps.append({
            "xT": xT16,
            "wq": np.ascontiguousarray(Wq[:, sl] * scale).astype(np.float16),
            "wkv": np.ascontiguousarray(
                np.concatenate([Wk[:, sl], Wv[:, sl]], axis=1)).astype(np.float16),
            "ptab": PTh,
            "cbias": np.ascontiguousarray(content_bias[c, 0, :, None]),
            "pbias": np.ascontiguousarray(pos_bias[c, 0, :, None]),
            "wout": np.ascontiguousarray(
                Wout_w[sl, :]).astype(ml_dtypes.bfloat16),
        })
    return in_maps


def kernel(x, Wq, Wk, Wv, content_bias, pos_bias, Wp_w, Wp_b, Wout_w, Wout_b):
    global _LAST_RESULT
    in_maps = _prep_inputs(x, Wq, Wk, Wv, content_bias, pos_bias,
                           Wp_w, Wp_b, Wout_w, Wout_b)
    nc = _build_graph()
    trace = bool(os.environ.get("KERNEL_TRACE"))
    res = run_bass_kernel_spmd(nc, in_maps, core_ids=list(range(NCORES)),
                               trace=trace, trace_cores=[0] if trace else None)
    _LAST_RESULT = res
    out = np.zeros((N, DM), np.float64)
    for c in range(NCORES):
        part = np.asarray(res.results[c]["out"], dtype=np.float64)  # (N, DM)
        rs = np.asarray(res.results[c]["rs"], dtype=np.float64)     # (1, N)
        out += part / rs[0][:, None]
    out += np.asarray(np.asarray(Wout_b, np.float32), np.float64)[None, :]
    return out[None].astype(np.float32)
